# revision 1
# baseline (speedup 1.0000x reference)
"""Trainium2 Bass kernel for retrieval-KNN soft attention (nn_NONA_54915451847255).

out = clip(softmax(-||x_i - x_n_j||_2, diag-masked) @ y_n, 0, 1)

Sharding: queries row-sharded across 8 cores; x_n / y_n replicated but ROLLED by
-core*1024 rows on the host so the self-match diagonal always falls in local key
tiles 0..7 -> the SPMD instruction stream is core-independent.

Math per core (1024 queries, 8192 keys), computed transposed (S_T[j,i]):
  psum = sum_d xnT[d,j] * (-2 x[d,i])  + 1*(qnorm_i - 512)      (PE, float32r)
  z    = psum + (knorm_j + 512)                                 (ACT bias / DVE on diag tiles)
  P_T  = exp(-exp(0.5 * ln(z)))  = exp(-sqrt(z))                (ACT, one table set)
  out_T[c,i] = sum_j y_aug[j,c] * P_T[j,i],  y_aug = [y_n | 1]  (PE, col C = rowsum)
  out[i,c] = clip(out_T[c,i] / out_T[C,i], 0, 1)
"""
import numpy as np

import concourse.bacc as bacc
import concourse.tile as tile
from concourse import mybir
from concourse.bass_utils import run_bass_kernel_spmd

F32 = mybir.dt.float32
F32R = mybir.dt.float32r
BF16 = mybir.dt.bfloat16
AF = mybir.ActivationFunctionType
ALU = mybir.AluOpType

N, D, C = 8192, 512, 100
NCORES = 8
QPC = N // NCORES          # 1024 queries per core
NKT = N // 128             # 64 key tiles
NQG = QPC // 512           # 2 query groups of 512
NDC = D // 128             # 4 contraction chunks
CA = C + 1                 # y augmented with ones column


_ACT_PATCHED = []


def _patch_act_tables():
    """Make Ln and Exp share one ACT LUT set (natural_log_exp_and_others).

    bacc's insert_act_table_loads picks, per ACTIVATE, a function-set from
    get_activation_tables() order; walrus remaps the set id positionally
    against its --act-root-json. Default order puts exp and ln in different
    sets -> a ~2.7us table reload per Ln<->Exp transition. Reorder both views
    consistently so natural_log_exp_and_others (contains ln AND exp) is
    first, and the loads hoist to a single ATL at kernel start.
    """
    if _ACT_PATCHED:
        return
    import json
    import os
    import tempfile

    import concourse.hw_specs as hw_specs
    import concourse.bacc as bacc_mod
    from neuronxcc.driver.Job import Job
    from neuronxcc.driver.jobs.support.FindActInfo import findActInfoFile

    FIRST = "natural_log_exp_and_others"
    src_json = findActInfoFile(Job.getPackageDir(), "gen3")
    src_dir = os.path.dirname(src_json)
    dst = tempfile.mkdtemp(prefix="act_override_")
    for f in os.listdir(src_dir):
        if f != "act_info.json":
            os.symlink(os.path.join(src_dir, f), os.path.join(dst, f))
    info = json.load(open(src_json))
    sets = info["act_func_sets"]
    sets.sort(key=lambda s: s["name"] != FIRST)
    with open(os.path.join(dst, "act_info.json"), "w") as f:
        json.dump(info, f)
    os.environ["BASS_ACT_ROOT_JSON_PATH"] = os.path.join(dst, "act_info.json")

    orig = hw_specs.get_activation_tables

    def patched(arch):
        d = orig(arch)
        items = sorted(d.items(), key=lambda kv: kv[0] != FIRST)
        return dict(items)

    hw_specs.get_activation_tables = patched
    bacc_mod.get_activation_tables = patched
    _ACT_PATCHED.append(True)


def build_nc(repeat=1):
    _patch_act_tables()
    nc = bacc.Bacc("TRN2", target_bir_lowering=False, debug=False)
    xq_d = nc.dram_tensor("xq", [QPC, D], F32, kind="ExternalInput").ap()
    xk_d = nc.dram_tensor("xk", [N, D], F32, kind="ExternalInput").ap()
    yk_d = nc.dram_tensor("yk", [N, C], F32, kind="ExternalInput").ap()
    id_d = nc.dram_tensor("ident", [128, 128], F32, kind="ExternalInput").ap()
    mk_d = nc.dram_tensor("dmask", [128, 128], F32, kind="ExternalInput").ap()
    out_d = nc.dram_tensor("out", [QPC, C], F32, kind="ExternalOutput").ap()

    with tile.TileContext(nc) as tc:
        with (
            tc.tile_pool(name="const", bufs=1) as constp,
            tc.tile_pool(name="ybank", bufs=1) as ybankp,
            tc.tile_pool(name="yraw", bufs=4) as yrawp,
            tc.tile_pool(name="xqraw", bufs=3) as xqrawp,
            tc.tile_pool(name="xt", bufs=1) as xtp,
            tc.tile_pool(name="xk", bufs=4) as xkp,
            tc.tile_pool(name="xnt", bufs=4) as xntp,
            tc.tile_pool(name="sq", bufs=3) as sqp,
            tc.tile_pool(name="kn", bufs=6) as knp,
            tc.tile_pool(name="act", bufs=4) as actp,
            tc.tile_pool(name="pt", bufs=4) as ptp,
            tc.tile_pool(name="fin", bufs=4) as finp,
            tc.tile_pool(name="trps", bufs=4, space="PSUM") as trps,
            tc.tile_pool(name="stps", bufs=2, space="PSUM") as stps,
            tc.tile_pool(name="outps", bufs=1, space="PSUM") as outps,
        ):
            ident = constp.tile([128, 128], F32)
            nc.sync.dma_start(ident[:], id_d)
            dmask = constp.tile([128, 128], BF16)
            dmask_f = constp.tile([128, 128], F32)
            nc.sync.dma_start(dmask_f[:], mk_d)
            nc.vector.tensor_copy(dmask[:], dmask_f[:])

            identb = constp.tile([128, 128], BF16)
            nc.vector.tensor_copy(identb[:], ident[:])
            ones2 = constp.tile([2, 128], BF16)
            nc.vector.memset(ones2[:], 1.0)

            for _rep in range(repeat):
                # ---- y bank: [128, 64*101] bf16, col 100 of each chunk = 1.0 ----
                ybank = ybankp.tile([128, NKT * CA], BF16)
                for t in range(NKT):
                    yr = yrawp.tile([128, C], F32)
                    nc.sync.dma_start(yr[:], yk_d[t * 128:(t + 1) * 128, :])
                    nc.vector.tensor_copy(ybank[:, t * CA:t * CA + C], yr[:])
                ones_col = ybank[:].rearrange("p (t c) -> p t c", c=CA)[:, :, C:CA]
                ones64 = constp.tile([128, NKT], F32)
                nc.vector.memset(ones64[:], 1.0)
                nc.vector.tensor_copy(ones_col, ones64[:].rearrange("p (t o) -> p t o", o=1))

                # ---- xT: [128, 4 * 1024] bf16 = -2 * x^T; qn2 = hi/lo bf16 of qnorm-512 ----
                # hi/lo split keeps the bf16 aug matmul at fp32-ish accuracy
                xT = xtp.tile([128, NDC * QPC], BF16)
                qn2 = constp.tile([2, QPC], BF16)
                for m in range(QPC // 128):
                    xqt = xqrawp.tile([128, D], F32)
                    nc.sync.dma_start(xqt[:], xq_d[m * 128:(m + 1) * 128, :])
                    sqt = sqp.tile([128, D], F32)
                    nc.vector.tensor_mul(sqt[:], xqt[:], xqt[:])
                    qn = knp.tile([128, 1], F32)
                    nc.vector.reduce_sum(qn[:], sqt[:], axis=mybir.AxisListType.X)
                    pair = knp.tile([128, 2], F32)
                    nc.vector.tensor_scalar_add(pair[:, 0:1], qn[:], -512.0)
                    hib = knp.tile([128, 1], BF16)
                    nc.vector.tensor_copy(hib[:], pair[:, 0:1])
                    hif = knp.tile([128, 1], F32)
                    nc.vector.tensor_copy(hif[:], hib[:])
                    nc.vector.tensor_sub(pair[:, 1:2], pair[:, 0:1], hif[:])
                    nc.vector.tensor_copy(pair[:, 0:1], hif[:])
                    ptr = trps.tile([2, 128], F32, tag="tr")
                    nc.tensor.transpose(ptr[:], pair[:], ident[:])
                    nc.vector.tensor_copy(qn2[:, m * 128:(m + 1) * 128], ptr[:])
                    xqb = xqrawp.tile([128, D], BF16)
                    nc.vector.tensor_scalar_mul(xqb[:], xqt[:], -2.0)
                    for kd in range(NDC):
                        ptx = trps.tile([128, 128], BF16, tag="tr")
                        nc.tensor.transpose(ptx[:], xqb[:, kd * 128:(kd + 1) * 128], identb[:])
                        nc.vector.tensor_copy(
                            xT[:, kd * QPC + m * 128: kd * QPC + (m + 1) * 128], ptx[:])

                # ---- persistent output accumulators [101, 512] per query group ----
                outp = [outps.tile([CA, 512], F32, name=f"outp{qg}") for qg in range(NQG)]

                # ---- main loop over key tiles ----
                for kt in range(NKT):
                    xkt = xkp.tile([128, D], F32)
                    nc.sync.dma_start(xkt[:], xk_d[kt * 128:(kt + 1) * 128, :])
                    sqt = sqp.tile([128, D], F32)
                    nc.vector.tensor_mul(sqt[:], xkt[:], xkt[:])
                    kn = knp.tile([128, 1], F32)
                    nc.vector.reduce_sum(kn[:], sqt[:], axis=mybir.AxisListType.X)
                    kb = knp.tile([128, 1], F32)
                    nc.vector.tensor_scalar_add(kb[:], kn[:], 512.0)

                    xkb = xkp.tile([128, D], BF16)
                    nc.vector.tensor_copy(xkb[:], xkt[:])
                    xnT = xntp.tile([128, D], BF16)
                    for kd in range(NDC):
                        ptx = trps.tile([128, 128], BF16, tag="tr")
                        nc.tensor.transpose(ptx[:], xkb[:, kd * 128:(kd + 1) * 128], identb[:])
                        nc.vector.tensor_copy(xnT[:, kd * 128:(kd + 1) * 128], ptx[:])

                    for qg in range(NQG):
                        st = stps.tile([128, 512], F32)
                        for kd in range(NDC):
                            nc.tensor.matmul(
                                st[:], xnT[:, kd * 128:(kd + 1) * 128],
                                xT[:, kd * QPC + qg * 512: kd * QPC + qg * 512 + 512],
                                start=(kd == 0), stop=False)
                        nc.tensor.matmul(
                            st[:], ones2[:], qn2[:, qg * 512:qg * 512 + 512],
                            start=False, stop=True)

                        diag = kt < 8 and qg == kt // 4
                        s1 = actp.tile([128, 512], F32)
                        if diag:
                            # z = psum + (knorm+512), clamped away from 0 under the diagonal
                            nc.vector.tensor_scalar(st[:], st[:], kb[:, 0:1], 350.0,
                                                    ALU.add, ALU.max)
                            nc.scalar.activation(s1[:], st[:], AF.Ln)
                        else:
                            nc.scalar.activation(s1[:], st[:], AF.Ln, bias=kb[:, 0:1])
                        s2 = actp.tile([128, 512], F32)
                        nc.scalar.activation(s2[:], s1[:], AF.Exp, scale=0.5)
                        pt = ptp.tile([128, 512], BF16)
                        nc.scalar.activation(pt[:], s2[:], AF.Exp, scale=-1.0)
                        if diag:
                            off = (kt % 4) * 128
                            nc.vector.tensor_mul(pt[:, off:off + 128],
                                                 pt[:, off:off + 128], dmask[:])
                        nc.tensor.matmul(outp[qg][:], ybank[:, kt * CA:(kt + 1) * CA],
                                         pt[:], start=(kt == 0), stop=(kt == NKT - 1))

                # ---- finalize: transpose back, normalize, clip, store ----
                for qg in range(NQG):
                    osb = finp.tile([CA, 512], F32)
                    nc.vector.tensor_copy(osb[:], outp[qg][:])
                    for t in range(4):
                        ptf = trps.tile([128, CA], F32, tag="tr")
                        nc.tensor.transpose(ptf[:], osb[:, t * 128:(t + 1) * 128],
                                            ident[0:CA, 0:CA])
                        rc = knp.tile([128, 1], F32)
                        nc.vector.reciprocal(rc[:], ptf[:, C:CA])
                        ob = finp.tile([128, C], F32)
                        nc.vector.tensor_scalar(ob[:], ptf[:, 0:C], rc[:, 0:1], 1.0,
                                                ALU.mult, ALU.min)
                        nc.sync.dma_start(
                            out_d[qg * 512 + t * 128: qg * 512 + (t + 1) * 128, :], ob[:])

    nc.compile()
    return nc


_NC_CACHE = []


def kernel(x, x_n, y_n):
    x = np.ascontiguousarray(np.asarray(x, dtype=np.float32))
    x_n = np.ascontiguousarray(np.asarray(x_n, dtype=np.float32))
    y_n = np.ascontiguousarray(np.asarray(y_n, dtype=np.float32))
    if not _NC_CACHE:
        _NC_CACHE.append(build_nc())
    nc = _NC_CACHE[0]

    ident = np.eye(128, dtype=np.float32)
    dmask = (1.0 - np.eye(128, dtype=np.float32))
    in_maps = []
    for c in range(NCORES):
        s = c * QPC
        in_maps.append({
            "xq": x[s:s + QPC],
            "xk": np.roll(x_n, -s, axis=0),
            "yk": np.roll(y_n, -s, axis=0),
            "ident": ident,
            "dmask": dmask,
        })
    import os
    trace = bool(int(os.environ.get("KERNEL_TRACE", "0")))
    res = run_bass_kernel_spmd(nc, in_maps, core_ids=list(range(NCORES)),
                               trace=trace)
    if trace:
        print("exec_time_ns:", res.exec_time_ns,
              "mean:", res.mean_exec_time_ns, flush=True)
        if res.instructions_and_trace:
            print("trace:", res.instructions_and_trace[1], flush=True)
    out = np.concatenate([r["out"] for r in res.results], axis=0)
    return out.astype(np.float32)

